# revision 1
# baseline (speedup 1.0000x reference)
"""Trainium2 Bass kernel for nn_DegreePrediction.

Computes y[u] = sum_{s,t,v} (x*W_t)[s,t] * (W_r*r_zeros + r_const)[s,t,u,v]
with N=80, streaming the three rank-4 tensors from HBM as fp16.

Sharding: leading s axis split across 8 cores (10 s-values = 800 (s,t) rows
per core, contiguous in DRAM). Each core computes partial outputs; partials
are summed on the host (the output is tiny, so no device collective).

Engine split, designed to sit at the DMA roofline (~30.7MB/core fp16 at
~350-420GB/s aggregate over the two DMA queues):

  DVE  comb16 = wr*rz          (fp16 2x mode, the only vector op per block)
  PE   psum[p:p+2, u, v] += l2pair[K,2].T @ comb16[K, chunk]  (start=False)
  PE   psum[p:p+2, u, v] += l2pair[K,2].T @ rc[K, chunk]      (same region!)
  DVE  one whole-tile PSUM->SBUF copy at the end; the v-reduction of the
       [u,v] accumulators happens on the host along with the cross-core sum

PSUM accumulation adds the rc and wr*rz streams for free. The contraction
over (s,t) runs in <=480-col chunks into 3 accumulator row-pairs at PSUM
partition bases 0/32/64 (the only legal matmul output bases), covering
u 0:27 / 27:54 / 54:80.

l2 = x*W_t is shipped as a Dekker pair (hi = fp16(l2), lo = fp16(l2-hi));
every matmul uses the [K,2] pair as stationary and the two output rows are
summed on the host, recovering f32-accurate l2 while keeping all matmul
operands fp16. PSUM accumulators are memset once and accumulated with
start=False: per-matmul start=True resets at PSUM *bank* granularity,
which clobbers neighbouring chunks sharing a bank.

Each tensor's per-block DMA is split across both queues (sync=SP engine,
scalar=Activation engine) because per-queue bandwidth (~180-200GB/s) bounds
a single tensor's arrival latency. The final block's mul is split into
column halves so its PE chunks can start after half its DMA has landed.
"""

import numpy as np

import concourse.bacc as bacc
import concourse.mybir as mybir
import concourse.tile as tile
from concourse.bass_utils import run_bass_kernel_spmd

N = 80
N_CORES = 8
S_PER_CORE = N // N_CORES            # 10
ST = S_PER_CORE * N                  # 800 (s,t) rows per core
N_BLOCKS = 7                         # 6*128 + 32
F32 = mybir.dt.float32
F16 = mybir.dt.float16

ROW_U = 27                           # max u values per psum row-pair
ROWS = [(0, 0, 27), (32, 27, 27), (64, 54, 26)]  # (partition, u0, u_count)


def _chunks(count):
    return [(s, min(6, count - s)) for s in range(0, count, 6)]


_CACHE = {}


def build_nc(repeats=1):
    nc = bacc.Bacc()
    wr_d = nc.declare_dram_parameter("wr", [ST, N, N], F16, isOutput=False)
    rz_d = nc.declare_dram_parameter("rz", [ST, N, N], F16, isOutput=False)
    rc_d = nc.declare_dram_parameter("rc", [ST, N * N], F16, isOutput=False)
    l2_d = nc.declare_dram_parameter("l2", [128, 2 * N_BLOCKS], F16, isOutput=False)
    y2_d = nc.declare_dram_parameter("y2", [2 * len(ROWS), ROW_U * N], F32, isOutput=True)

    with tile.TileContext(nc) as tc:
        with (
            tc.tile_pool(name="io", bufs=2) as pool,
            tc.tile_pool(name="small", bufs=1) as sp,
            tc.psum_pool(name="ps", bufs=1) as pp,
        ):
            l2_sb = sp.tile([128, 2 * N_BLOCKS], F16)
            nc.sync.dma_start(out=l2_sb[:], in_=l2_d[:])
            psum2 = pp.tile([66, ROW_U, N], F32)
            nc.vector.memset(psum2[:], 0.0)

            for r in range(repeats):
                for b in range(N_BLOCKS):
                    r0 = b * 128
                    K = min(128, ST - r0)
                    wr_t = pool.tile([128, N, N], F16, tag="wr", bufs=4)
                    rz_t = pool.tile([128, N, N], F16, tag="rz", bufs=4)
                    rc_t = pool.tile([128, N * N], F16, tag="rc", bufs=4)
                    h = N // 2
                    nc.sync.dma_start(out=wr_t[:K, 0:h, :], in_=wr_d[r0 : r0 + K, 0:h, :])
                    nc.scalar.dma_start(out=wr_t[:K, h:N, :], in_=wr_d[r0 : r0 + K, h:N, :])
                    nc.sync.dma_start(out=rz_t[:K, 0:h, :], in_=rz_d[r0 : r0 + K, 0:h, :])
                    nc.scalar.dma_start(out=rz_t[:K, h:N, :], in_=rz_d[r0 : r0 + K, h:N, :])
                    hf = N * N // 2
                    nc.sync.dma_start(out=rc_t[:K, 0:hf], in_=rc_d[r0 : r0 + K, 0:hf])
                    nc.scalar.dma_start(out=rc_t[:K, hf:], in_=rc_d[r0 : r0 + K, hf:])

                    last = (r == repeats - 1) and (b == N_BLOCKS - 1)
                    l2p = l2_sb[0:K, 2 * b : 2 * b + 2]

                    comb16 = pool.tile([128, N, N], F16, tag="comb")
                    if last:
                        # split the final mul so the drain starts after half
                        # the block's wr/rz DMA has landed
                        nc.vector.tensor_mul(
                            out=comb16[:K, 0:h, :], in0=wr_t[:K, 0:h, :], in1=rz_t[:K, 0:h, :]
                        )
                        nc.vector.tensor_mul(
                            out=comb16[:K, h:N, :], in0=wr_t[:K, h:N, :], in1=rz_t[:K, h:N, :]
                        )
                    else:
                        nc.vector.tensor_mul(out=comb16[:K], in0=wr_t[:K], in1=rz_t[:K])

                    for p, ubase, ucount in ROWS:
                        for src in (comb16, rc_t):
                            for u0, un in _chunks(ucount):
                                ua = ubase + u0
                                rhs = (
                                    comb16[:K, ua : ua + un, :]
                                    if src is comb16
                                    else rc_t[:K, ua * N : (ua + un) * N]
                                )
                                nc.tensor.matmul(
                                    psum2[p : p + 2, u0 : u0 + un, :],
                                    l2p,
                                    rhs,
                                    start=False,
                                    stop=last and src is rc_t and u0 + un == ucount,
                                    skip_group_check=True,
                                )

            # ship the un-reduced accumulators; the v-sum joins the host-side
            # cross-core reduction
            y2_sb = sp.tile([66, ROW_U * N], F32)
            nc.vector.tensor_copy(out=y2_sb[:], in_=psum2[:])
            for row, (p, ubase, ucount) in enumerate(ROWS):
                nc.sync.dma_start(
                    out=y2_d[2 * row : 2 * row + 2, :], in_=y2_sb[p : p + 2, :]
                )
    nc.compile()
    return nc


def _get_nc():
    if "nc" not in _CACHE:
        _CACHE["nc"] = build_nc()
    return _CACHE["nc"]


def make_in_maps(x, r_zeros, r_const, weights_t, weights_r):
    l2 = np.asarray(x, np.float32) * np.asarray(weights_t, np.float32)
    wr16 = np.asarray(weights_r, np.float32).astype(np.float16)
    rz16 = np.asarray(r_zeros, np.float32).astype(np.float16)
    rc16 = np.asarray(r_const, np.float32).astype(np.float16)
    in_maps = []
    for c in range(N_CORES):
        sl = slice(c * S_PER_CORE, (c + 1) * S_PER_CORE)
        l2p = np.zeros(128 * N_BLOCKS, np.float32)
        l2p[:ST] = l2[sl].reshape(-1)
        blk = l2p.reshape(N_BLOCKS, 128)
        hi = blk.astype(np.float16)
        lo = (blk - hi.astype(np.float32)).astype(np.float16)
        l2cols = np.empty((128, 2 * N_BLOCKS), np.float16)
        l2cols[:, 0::2] = hi.T
        l2cols[:, 1::2] = lo.T
        in_maps.append(
            {
                "wr": wr16[sl].reshape(ST, N, N),
                "rz": rz16[sl].reshape(ST, N, N),
                "rc": rc16[sl].reshape(ST, N * N),
                "l2": np.ascontiguousarray(l2cols),
            }
        )
    return in_maps


def run(x, r_zeros, r_const, weights_t, weights_r, **spmd_kwargs):
    nc = _get_nc()
    in_maps = make_in_maps(x, r_zeros, r_const, weights_t, weights_r)
    res = run_bass_kernel_spmd(nc, in_maps, list(range(N_CORES)), **spmd_kwargs)
    y = np.zeros(N, np.float32)
    for i in range(N_CORES):
        y2 = res.results[i]["y2"].reshape(len(ROWS), 2, ROW_U, N)
        part = y2.sum(axis=(1, 3))  # dekker-pair rows + v
        for row, (_, ubase, ucount) in enumerate(ROWS):
            y[ubase : ubase + ucount] += part[row, 0:ucount]
    return y, res


def kernel(x, r_zeros, r_const, weights_t, weights_r):
    y, _ = run(x, r_zeros, r_const, weights_t, weights_r)
    return y

